# revision 19
# baseline (speedup 1.0000x reference)
"""CIN (xDeepFM Compressed Interaction Network) Trainium2 kernel.

Problem: B=256, M=256, D=16, H1=H2=64, HN=32.
  X0[b,m,d] = x[b,m] * emb[m,d]
  X1 = relu(einsum('bhd,bmd,ohm->bod', X0, X0, W0r) + b0)   W0r=[64,256,256]
  X2 = relu(einsum('bhd,bmd,ohm->bod', Xp, X0, W1r) + b1)   Xp=X1[:,:32,:]
  out = concat(X1[:,32:].sum(d), X2.sum(d)) @ fcW.T + fcb

Sharding: data-parallel over batch, 32 batches per core on 8 cores.
Per-core device algorithm (bd = 512 columns, col = d*32 + b_local):
  v[m,(d,b)]    = x[b,m]*emb[m,d]   (fp32r, lhsT of all matmuls)
  vT[(d,b), m]  = same values, [bd-partition, m-free] layout, 4 slots
  U^T[bd,(o,h)] = PE fp32r matmuls into paired 2-bank PSUM tiles
  step 2        = DVE multiply (U^T * vT broadcast) into an SBUF tmp, then
                  ACT activation(Copy, accum_out=) sums each 256-wide h-group
  layer 2 analogous (h2-groups of 32, reduced on DVE); d-summation via PE
  matmul with a stacked identity; final 96->2 fc on DVE.
"""

import numpy as np

import concourse.bass as bass
import concourse.mybir as mybir
import concourse.tile as tile
from concourse import bacc
from concourse.bass_utils import run_bass_kernel_spmd

B, M, D = 256, 256, 16
H1, H2 = 64, 64
HN = H1 // 2
N_CORES = 8
BL = B // N_CORES          # 32 batches per core
BD = BL * D                # 512 columns per core
OUTW = (H1 - HN) + H2      # 96

F32 = mybir.dt.float32
F32R = mybir.dt.float32r

_CACHE = {}


def _build_nc():
    nc = bacc.Bacc("TRN2", target_bir_lowering=False, debug=False)

    at = nc.dram_tensor("at", [M, H1 * M], F32R, kind="ExternalInput")      # [m,(o,h)]
    w1t = nc.dram_tensor("w1t", [M, H2 * HN], F32R, kind="ExternalInput")   # [m,(o2,h2)]
    emb_d = nc.dram_tensor("emb", [128, 2 * D], F32R, kind="ExternalInput")     # [p,(ko d)]
    embt_d = nc.dram_tensor("embt", [D, M], F32, kind="ExternalInput")
    xt_d = nc.dram_tensor("xt", [128, 2 * BL], F32R, kind="ExternalInput")       # [p,(ko b)]
    xs_d = nc.dram_tensor("xs", [BL, M], F32, kind="ExternalInput")              # x shard
    sel_d = nc.dram_tensor("sel", [D, 4 * 128], F32, kind="ExternalInput")       # er selector
    ones_d = nc.dram_tensor("ones", [1, 128], F32, kind="ExternalInput")
    b0_d = nc.dram_tensor("b0", [1, H1], F32, kind="ExternalInput")
    b1_d = nc.dram_tensor("b1", [1, H2], F32, kind="ExternalInput")
    fcw_d = nc.dram_tensor("fcw", [2, OUTW], F32, kind="ExternalInput")
    fcb_d = nc.dram_tensor("fcb", [1, 2], F32, kind="ExternalInput")
    eye_d = nc.dram_tensor("eye32", [BL, BL], F32, kind="ExternalInput")
    y_d = nc.dram_tensor("y", [BL, 2], F32, kind="ExternalOutput")

    at3 = at.rearrange("(ko p) c -> p ko c", p=128)
    w1t3 = w1t.rearrange("(ko p) c -> p ko c", p=128)

    NJ = H1 // 2            # 32 layer-1 col chunks (2 output ch each)
    NJJ = (H2 * HN) // 512  # 4 layer-2 col chunks (16 output ch each)

    with tile.TileContext(nc) as tc:
        with (
            tc.tile_pool(name="const", bufs=1) as cpool,
            tc.tile_pool(name="achunk", bufs=6) as apool,
            tc.tile_pool(name="scr", bufs=6) as spool,
            tc.tile_pool(name="dump", bufs=6) as dpool,
            tc.tile_pool(name="psum", bufs=3, space="PSUM") as ppool,
        ):
            # ---- constants / activations prep (small contiguous DMAs only;
            # partition replication done via PE matmuls) ----
            xt_sb = cpool.tile([128, 2, BL], F32R)
            nc.sync.dma_start(xt_sb[:], xt_d.rearrange("p (ko b) -> p ko b", ko=2))
            emb_sb = cpool.tile([128, 2, D], F32R)
            nc.sync.dma_start(emb_sb[:], emb_d.rearrange("p (ko d) -> p ko d", ko=2))
            xs_sb = cpool.tile([BL, M], F32)
            nc.sync.dma_start(xs_sb[:], xs_d[:])
            embt_sb = cpool.tile([D, M], F32)
            nc.sync.dma_start(embt_sb[:], embt_d[:])
            sel_sb = cpool.tile([D, 4, 128], F32)
            nc.sync.dma_start(sel_sb[:], sel_d.rearrange("d (t p) -> d t p", t=4))
            ones_sb = cpool.tile([1, 128], F32)
            nc.sync.dma_start(ones_sb[:], ones_d[:])
            eye_sb = cpool.tile([BL, BL], F32)
            nc.sync.dma_start(eye_sb[:], eye_d[:])
            b0_sb = cpool.tile([1, H1], F32)
            nc.sync.dma_start(b0_sb[:], b0_d[:])
            b1_sb = cpool.tile([1, H2], F32)
            nc.sync.dma_start(b1_sb[:], b1_d[:])
            fcw_sb = cpool.tile([1, 2 * OUTW], F32)
            nc.sync.dma_start(fcw_sb[:], fcw_d.rearrange("c k -> (c k)")[None, :])
            fcb_sb = cpool.tile([1, 2], F32)
            nc.sync.dma_start(fcb_sb[:], fcb_d[:])

            # v[m,(d,b)] as [128, ko, d, b]
            v = cpool.tile([128, 2, D, BL], F32R)
            nc.vector.tensor_tensor(
                out=v[:],
                in0=xt_sb[:, :, None, :].to_broadcast([128, 2, D, BL]),
                in1=emb_sb[:, :, :, None].to_broadcast([128, 2, D, BL]),
                op=mybir.AluOpType.mult,
            )

            # PE-based replications: rep = eye32 tiled 4x along M
            rep_sb = cpool.tile([BL, 4 * BL], F32)
            nc.vector.tensor_copy(
                rep_sb.rearrange("p (t b) -> p t b", t=4),
                eye_sb[:, None, :].to_broadcast([BL, 4, BL]),
            )
            rep_ap = rep_sb[:]  # [32, 128]
            xe_ps = ppool.tile([128, 2, 512], F32, tag="u", name="xe_ps")
            nc.tensor.matmul(xe_ps[:, 0, :256], rep_ap, xs_sb[:], start=True, stop=True)
            # er[p=(dl,b), t, m] = embt[4t+dl, m] via selector matmuls
            er_ps = ppool.tile([128, 2, 512], F32, tag="u", name="er_ps")
            for t in range(4):
                nc.tensor.matmul(
                    er_ps[:, t // 2, 256 * (t % 2) : 256 * (t % 2) + 256],
                    sel_sb[:, t, :],
                    embt_sb[:],
                    start=True,
                    stop=True,
                )
            # vT[(d,b), t, m] = xe * er  (xe to SBUF first: one PSUM input max)
            xe_sb = cpool.tile([128, M], F32)
            nc.scalar.copy(xe_sb[:], xe_ps[:, 0, :256])
            vT = cpool.tile([128, 4, M], F32)
            nc.vector.tensor_tensor(
                out=vT[:],
                in0=xe_sb[:, None, :].to_broadcast([128, 4, 256]),
                in1=er_ps.rearrange("p b (tt m) -> p (b tt) m", m=256),
                op=mybir.AluOpType.mult,
            )

            # replicated per-partition constants via ones/rep matmuls
            cr_ps = ppool.tile([128, 2, 512], F32, tag="u", name="cr_ps")
            nc.tensor.matmul(cr_ps[:, 0, 0:H1], ones_sb[:], b0_sb[:], start=True, stop=True)
            nc.tensor.matmul(cr_ps[:, 0, H1 : H1 + H2], ones_sb[:], b1_sb[:], start=True, stop=True)
            nc.tensor.matmul(
                cr_ps[:32, 0, 128 : 128 + 2 * OUTW],
                ones_sb[:, :32],
                fcw_sb[:],
                start=True,
                stop=True,
            )
            nc.tensor.matmul(
                cr_ps[:32, 0, 320:322], ones_sb[:, :32], fcb_sb[:], start=True, stop=True
            )
            nc.tensor.matmul(cr_ps[:, 1, :BL], rep_ap, eye_sb[:], start=True, stop=True)
            b0r = cpool.tile([128, H1], F32)
            nc.scalar.copy(b0r[:], cr_ps[:, 0, 0:H1])
            b1r = cpool.tile([128, H2], F32)
            nc.scalar.copy(b1r[:], cr_ps[:, 0, H1 : H1 + H2])
            fcwr = cpool.tile([BL, 2, OUTW], F32)
            nc.scalar.copy(fcwr[:], cr_ps[:32, 0, 128 : 128 + 2 * OUTW].rearrange("p (c k) -> p c k", c=2))
            fcbr = cpool.tile([BL, 2], F32)
            nc.scalar.copy(fcbr[:], cr_ps[:32, 0, 320:322])
            s4 = cpool.tile([128, BL], F32)
            nc.scalar.copy(s4[:], cr_ps[:, 1, :BL])

            # accumulation targets: slot t = bd-tile t
            xpc = cpool.tile([128, 4, HN], F32)
            ctc = cpool.tile([128, 4, OUTW], F32)

            # PE warm-up: dummy matmuls while the first weight chunks stream
            # in, so the HAM clock-gate reaches 8/8 before real work starts.
            warm_w = cpool.tile([128, 128], F32)
            warm_r = cpool.tile([128, 512], F32)
            nc.gpsimd.memset(warm_w[:], 0.0)
            nc.gpsimd.memset(warm_r[:], 0.0)
            for _w in range(4):
                warm_ps = ppool.tile([128, 2, 512], F32, tag="u", name="warm_ps")
                for _ in range(4):
                    nc.tensor.matmul(
                        warm_ps[:, 0, :], warm_w[:], warm_r[:], start=True, stop=True
                    )

            def lhs(t, ko):
                # stationary operand [m-chunk 128, 128 bd cols of tile t]
                return v[:, ko, 4 * t : 4 * (t + 1), :]

            # ---- layer 1 + interleaved layer 2 ----
            def l1_step(j):
                a_j = apool.tile([128, 2, 512], F32R, tag="a", name="a_j")
                nc.sync.dma_start(
                    a_j[:, 0, 0:384], at3[:, 0, 512 * j : 512 * j + 384]
                )
                nc.sync.dma_start(a_j[:, 1, :], at3[:, 1, 512 * j : 512 * (j + 1)])
                for tp in range(2):  # tile pair (2*tp, 2*tp+1)
                    ps = ppool.tile([128, 2, 512], F32, tag="u", name="ps")
                    for b_ in range(2):
                        t = 2 * tp + b_
                        # triangular-packed weights: ko=0 rows only touch
                        # h<192... columns [0:384] (rest are zeros). Full-width
                        # ko=1 matmul first so every element gets a start=True
                        # write, then the narrower ko=0 accumulate.
                        nc.tensor.matmul(
                            ps[:, b_, :], lhs(t, 1), a_j[:, 1, :],
                            start=True, stop=False,
                        )
                        nc.tensor.matmul(
                            ps[:, b_, 0:384], lhs(t, 0), a_j[:, 0, 0:384],
                            start=False, stop=True, skip_group_check=True,
                        )
                    tmp = spool.tile([128, 2, 2, 256], F32, tag="tmp", name="tmp")
                    nc.vector.tensor_tensor(
                        out=tmp[:],
                        in0=ps.rearrange("p b (oh m) -> p b oh m", m=256),
                        in1=vT[:, 2 * tp : 2 * tp + 2, None, :].to_broadcast(
                            [128, 2, 2, 256]
                        ),
                        op=mybir.AluOpType.mult,
                    )
                    if (j % 3) == 2 or j in (30, 31):
                        # DVE grouped reduce: out[b_, oh] -> (t=2tp+b_, o=2j+oh)
                        tgt4 = (
                            xpc[:, 2 * tp : 2 * tp + 2, 2 * j : 2 * j + 2]
                            if 2 * j < HN
                            else ctc[:, 2 * tp : 2 * tp + 2, 2 * j - HN : 2 * j - HN + 2]
                        )
                        nc.vector.tensor_reduce(
                            out=tgt4,
                            in_=tmp[:],
                            axis=mybir.AxisListType.X,
                            op=mybir.AluOpType.add,
                        )
                    else:
                        for b_ in range(2):
                            t = 2 * tp + b_
                            for oh in range(2):
                                o = 2 * j + oh
                                tgt = (
                                    xpc[:, t, o : o + 1]
                                    if o < HN
                                    else ctc[:, t, o - HN : o - HN + 1]
                                )
                                dump = dpool.tile([128, 256], F32, tag="dump", name="dump")
                                nc.scalar.activation(
                                    dump[:],
                                    tmp[:, b_, oh, :],
                                    mybir.ActivationFunctionType.Copy,
                                    bias=0.0,
                                    scale=1.0,
                                    accum_out=tgt,
                                )

            def l2_step(jj):
                w_jj = apool.tile([128, 2, 512], F32R, tag="a", name="w_jj")
                nc.sync.dma_start(w_jj[:], w1t3[:, :, 512 * jj : 512 * (jj + 1)])
                for tp in range(2):
                    ps2 = ppool.tile([128, 2, 512], F32, tag="u", name="ps2")
                    for b_ in range(2):
                        t = 2 * tp + b_
                        for ko in range(2):
                            nc.tensor.matmul(
                                ps2[:, b_, :],
                                lhs(t, ko),
                                w_jj[:, ko, :],
                                start=(ko == 0),
                                stop=(ko == 1),
                            )
                    tmp2 = spool.tile([128, 2, 16, HN], F32, tag="tmp2", name="tmp2")
                    nc.vector.tensor_tensor(
                        out=tmp2[:],
                        in0=ps2.rearrange("p b (g h) -> p b g h", h=HN),
                        in1=xpc[:, 2 * tp : 2 * tp + 2, None, :].to_broadcast(
                            [128, 2, 16, HN]
                        ),
                        op=mybir.AluOpType.mult,
                    )
                    for b_ in range(2):
                        t = 2 * tp + b_
                        nc.vector.tensor_reduce(
                            out=ctc[:, t, HN + 16 * jj : HN + 16 * (jj + 1)],
                            in_=tmp2[:, b_],
                            axis=mybir.AxisListType.X,
                            op=mybir.AluOpType.add,
                        )

            for j in range(NJ // 2):
                l1_step(j)
            # hidden half accumulated -> bias + relu Xp for layer 2
            nc.vector.tensor_tensor(
                out=xpc[:],
                in0=xpc[:],
                in1=b0r[:, None, :HN].to_broadcast([128, 4, HN]),
                op=mybir.AluOpType.add,
            )
            nc.vector.tensor_scalar_max(xpc[:], xpc[:], 0.0)
            for j in range(NJ // 2, NJ):
                l1_step(j)
                if j % 4 == 1:
                    l2_step((j - NJ // 2) // 4)

            # bias + relu: ctc cols 0:32 are X1[32:64] (need b0[32:]), cols
            # 32:96 are X2 (need b1)
            nc.vector.tensor_tensor(
                out=ctc[:, :, :HN],
                in0=ctc[:, :, :HN],
                in1=b0r[:, None, HN:].to_broadcast([128, 4, HN]),
                op=mybir.AluOpType.add,
            )
            nc.vector.tensor_tensor(
                out=ctc[:, :, HN:],
                in0=ctc[:, :, HN:],
                in1=b1r[:, None, :].to_broadcast([128, 4, H2]),
                op=mybir.AluOpType.add,
            )
            nc.vector.tensor_scalar_max(ctc[:], ctc[:], 0.0)

            # ---- d-sum + fc ----
            psf = ppool.tile([BL, OUTW], F32, tag="u", name="psf")
            for t in range(4):
                nc.tensor.matmul(psf[:], s4[:], ctc[:, t, :], start=(t == 0), stop=(t == 3))
            cin = spool.tile([BL, OUTW], F32, tag="cin")
            nc.scalar.copy(cin[:], psf[:])
            y_sb = spool.tile([BL, 2], F32, tag="ysb")
            prod = spool.tile([BL, 2, OUTW], F32, tag="prod")
            nc.vector.tensor_tensor(
                out=prod[:],
                in0=cin[:, None, :].to_broadcast([BL, 2, OUTW]),
                in1=fcwr[:],
                op=mybir.AluOpType.mult,
            )
            nc.vector.tensor_reduce(
                out=y_sb[:],
                in_=prod[:],
                axis=mybir.AxisListType.X,
                op=mybir.AluOpType.add,
            )
            nc.vector.tensor_tensor(
                out=y_sb[:], in0=y_sb[:], in1=fcbr[:], op=mybir.AluOpType.add
            )
            nc.sync.dma_start(y_d[:], y_sb[:])

    nc.finalize()
    return nc


def kernel(x, emb, W0, b0, W1, b1, fcW, fcb):
    x = np.ascontiguousarray(x, dtype=np.float32)
    emb = np.ascontiguousarray(emb, dtype=np.float32)

    # host-side: symmetrize the quadratic form and pack upper-triangular
    # (zero for m < h, doubled off-diagonal), then permute [o,h,m] -> [m,(o,h)]
    W0r_ = W0.reshape(H1, M, M).astype(np.float64)
    S = 0.5 * (W0r_ + W0r_.transpose(0, 2, 1))
    iu = np.triu_indices(M, 1)
    Tri = np.zeros_like(S)
    Tri[:, np.arange(M), np.arange(M)] = S[:, np.arange(M), np.arange(M)]
    Tri[:, iu[0], iu[1]] = 2.0 * S[:, iu[0], iu[1]]
    at = np.ascontiguousarray(
        Tri.transpose(2, 0, 1).reshape(M, H1 * M).astype(np.float32)
    )
    w1t = np.ascontiguousarray(
        W1.reshape(H2, HN, M).transpose(2, 0, 1).reshape(M, H2 * HN).astype(np.float32)
    )
    embt = np.ascontiguousarray(emb.T)
    eye32 = np.eye(BL, dtype=np.float32)
    emb_arr = np.ascontiguousarray(
        emb.reshape(2, 128, D).transpose(1, 0, 2).reshape(128, 2 * D)
    )
    sel = np.zeros((D, 4, 128), dtype=np.float32)
    for t in range(4):
        for p in range(128):
            sel[4 * t + p // 32, t, p] = 1.0
    sel = sel.reshape(D, 4 * 128)
    ones = np.ones((1, 128), dtype=np.float32)

    shared = {
        "at": at,
        "w1t": w1t,
        "emb": emb_arr,
        "embt": embt,
        "sel": sel,
        "ones": ones,
        "b0": np.ascontiguousarray(b0.reshape(1, H1).astype(np.float32)),
        "b1": np.ascontiguousarray(b1.reshape(1, H2).astype(np.float32)),
        "fcw": np.ascontiguousarray(fcW.astype(np.float32)),
        "fcb": np.ascontiguousarray(fcb.reshape(1, 2).astype(np.float32)),
        "eye32": eye32,
    }
    in_maps = []
    for c in range(N_CORES):
        xs = np.ascontiguousarray(x[BL * c : BL * (c + 1)])
        m = dict(shared)
        m["xs"] = xs
        m["xt"] = np.ascontiguousarray(
            xs.T.reshape(2, 128, BL).transpose(1, 0, 2).reshape(128, 2 * BL)
        )
        in_maps.append(m)

    if "nc" not in _CACHE:
        _CACHE["nc"] = _build_nc()
    global _last_in_maps
    _last_in_maps = in_maps
    res = run_bass_kernel_spmd(_CACHE["nc"], in_maps, core_ids=list(range(N_CORES)))
    return np.concatenate([r["y"] for r in res.results], axis=0)


# revision 20
# speedup vs baseline: 1.0916x; 1.0916x over previous
"""CIN (xDeepFM Compressed Interaction Network) Trainium2 kernel.

Problem: B=256, M=256, D=16, H1=H2=64, HN=32.
  X0[b,m,d] = x[b,m] * emb[m,d]
  X1 = relu(einsum('bhd,bmd,ohm->bod', X0, X0, W0r) + b0)   W0r=[64,256,256]
  X2 = relu(einsum('bhd,bmd,ohm->bod', Xp, X0, W1r) + b1)   Xp=X1[:,:32,:]
  out = concat(X1[:,32:].sum(d), X2.sum(d)) @ fcW.T + fcb

Sharding: data-parallel over batch, 32 batches per core on 8 cores.
Per-core device algorithm (bd = 512 columns, col = d*32 + b_local):
  v[m,(d,b)]    = x[b,m]*emb[m,d]   (fp32r, lhsT of all matmuls)
  vT[(d,b), m]  = same values, [bd-partition, m-free] layout, 4 slots
  U^T[bd,(o,h)] = PE fp32r matmuls into paired 2-bank PSUM tiles
  step 2        = DVE multiply (U^T * vT broadcast) into an SBUF tmp, then
                  ACT activation(Copy, accum_out=) sums each 256-wide h-group
  layer 2 analogous (h2-groups of 32, reduced on DVE); d-summation via PE
  matmul with a stacked identity; final 96->2 fc on DVE.
"""

import numpy as np

import concourse.bass as bass
import concourse.mybir as mybir
import concourse.tile as tile
from concourse import bacc
from concourse.bass_utils import run_bass_kernel_spmd

B, M, D = 256, 256, 16
H1, H2 = 64, 64
HN = H1 // 2
N_CORES = 8
BL = B // N_CORES          # 32 batches per core
BD = BL * D                # 512 columns per core
OUTW = (H1 - HN) + H2      # 96

F32 = mybir.dt.float32
F32R = mybir.dt.float32r

_CACHE = {}


def _build_nc():
    nc = bacc.Bacc("TRN2", target_bir_lowering=False, debug=False)

    at = nc.dram_tensor("at", [M, H1 * M], F32R, kind="ExternalInput")      # [m,(o,h)]
    w1t = nc.dram_tensor("w1t", [M, H2 * HN], F32R, kind="ExternalInput")   # [m,(o2,h2)]
    emb_d = nc.dram_tensor("emb", [128, 2 * D], F32R, kind="ExternalInput")     # [p,(ko d)]
    embt_d = nc.dram_tensor("embt", [D, M], F32, kind="ExternalInput")
    xt_d = nc.dram_tensor("xt", [128, 2 * BL], F32R, kind="ExternalInput")       # [p,(ko b)]
    xs_d = nc.dram_tensor("xs", [BL, M], F32, kind="ExternalInput")              # x shard
    sel_d = nc.dram_tensor("sel", [D, 4 * 128], F32, kind="ExternalInput")       # er selector
    ones_d = nc.dram_tensor("ones", [1, 128], F32, kind="ExternalInput")
    b0_d = nc.dram_tensor("b0", [1, H1], F32, kind="ExternalInput")
    b1_d = nc.dram_tensor("b1", [1, H2], F32, kind="ExternalInput")
    fcw_d = nc.dram_tensor("fcw", [2, OUTW], F32, kind="ExternalInput")
    fcb_d = nc.dram_tensor("fcb", [1, 2], F32, kind="ExternalInput")
    eye_d = nc.dram_tensor("eye32", [BL, BL], F32, kind="ExternalInput")
    y_d = nc.dram_tensor("y", [BL, 2], F32, kind="ExternalOutput")

    at3 = at.rearrange("(ko p) c -> p ko c", p=128)
    w1t3 = w1t.rearrange("(ko p) c -> p ko c", p=128)

    NJ = H1 // 2            # 32 layer-1 col chunks (2 output ch each)
    NJJ = (H2 * HN) // 512  # 4 layer-2 col chunks (16 output ch each)

    with tile.TileContext(nc) as tc:
        with (
            tc.tile_pool(name="const", bufs=1) as cpool,
            tc.tile_pool(name="achunk", bufs=6) as apool,
            tc.tile_pool(name="scr", bufs=6) as spool,
            tc.tile_pool(name="dump", bufs=6) as dpool,
            tc.tile_pool(name="psum", bufs=3, space="PSUM") as ppool,
        ):
            # ---- constants / activations prep (small contiguous DMAs only;
            # partition replication done via PE matmuls) ----
            xt_sb = cpool.tile([128, 2, BL], F32R)
            nc.sync.dma_start(xt_sb[:], xt_d.rearrange("p (ko b) -> p ko b", ko=2))
            emb_sb = cpool.tile([128, 2, D], F32R)
            nc.sync.dma_start(emb_sb[:], emb_d.rearrange("p (ko d) -> p ko d", ko=2))
            xs_sb = cpool.tile([BL, M], F32)
            nc.sync.dma_start(xs_sb[:], xs_d[:])
            embt_sb = cpool.tile([D, M], F32)
            nc.sync.dma_start(embt_sb[:], embt_d[:])
            sel_sb = cpool.tile([D, 4, 128], F32)
            nc.sync.dma_start(sel_sb[:], sel_d.rearrange("d (t p) -> d t p", t=4))
            ones_sb = cpool.tile([1, 128], F32)
            nc.sync.dma_start(ones_sb[:], ones_d[:])
            eye_sb = cpool.tile([BL, BL], F32)
            nc.sync.dma_start(eye_sb[:], eye_d[:])
            b0_sb = cpool.tile([1, H1], F32)
            nc.sync.dma_start(b0_sb[:], b0_d[:])
            b1_sb = cpool.tile([1, H2], F32)
            nc.sync.dma_start(b1_sb[:], b1_d[:])
            fcw_sb = cpool.tile([1, 2 * OUTW], F32)
            nc.sync.dma_start(fcw_sb[:], fcw_d.rearrange("c k -> (c k)")[None, :])
            fcb_sb = cpool.tile([1, 2], F32)
            nc.sync.dma_start(fcb_sb[:], fcb_d[:])

            # v[m,(d,b)] as [128, ko, d, b]
            v = cpool.tile([128, 2, D, BL], F32R)
            nc.vector.tensor_tensor(
                out=v[:],
                in0=xt_sb[:, :, None, :].to_broadcast([128, 2, D, BL]),
                in1=emb_sb[:, :, :, None].to_broadcast([128, 2, D, BL]),
                op=mybir.AluOpType.mult,
            )

            # PE-based replications: rep = eye32 tiled 4x along M
            rep_sb = cpool.tile([BL, 4 * BL], F32)
            nc.vector.tensor_copy(
                rep_sb.rearrange("p (t b) -> p t b", t=4),
                eye_sb[:, None, :].to_broadcast([BL, 4, BL]),
            )
            rep_ap = rep_sb[:]  # [32, 128]
            xe_ps = ppool.tile([128, 2, 512], F32, tag="u", name="xe_ps")
            nc.tensor.matmul(xe_ps[:, 0, :256], rep_ap, xs_sb[:], start=True, stop=True)
            # er[p=(dl,b), t, m] = embt[4t+dl, m] via selector matmuls
            er_ps = ppool.tile([128, 2, 512], F32, tag="u", name="er_ps")
            for t in range(4):
                nc.tensor.matmul(
                    er_ps[:, t // 2, 256 * (t % 2) : 256 * (t % 2) + 256],
                    sel_sb[:, t, :],
                    embt_sb[:],
                    start=True,
                    stop=True,
                )
            # vT[(d,b), t, m] = xe * er  (xe to SBUF first: one PSUM input max)
            xe_sb = cpool.tile([128, M], F32)
            nc.scalar.copy(xe_sb[:], xe_ps[:, 0, :256])
            vT = cpool.tile([128, 4, M], F32)
            nc.vector.tensor_tensor(
                out=vT[:],
                in0=xe_sb[:, None, :].to_broadcast([128, 4, 256]),
                in1=er_ps.rearrange("p b (tt m) -> p (b tt) m", m=256),
                op=mybir.AluOpType.mult,
            )

            # replicated per-partition constants via ones/rep matmuls
            cr_ps = ppool.tile([128, 2, 512], F32, tag="u", name="cr_ps")
            nc.tensor.matmul(cr_ps[:, 0, 0:H1], ones_sb[:], b0_sb[:], start=True, stop=True)
            nc.tensor.matmul(cr_ps[:, 0, H1 : H1 + H2], ones_sb[:], b1_sb[:], start=True, stop=True)
            nc.tensor.matmul(
                cr_ps[:32, 0, 128 : 128 + 2 * OUTW],
                ones_sb[:, :32],
                fcw_sb[:],
                start=True,
                stop=True,
            )
            nc.tensor.matmul(
                cr_ps[:32, 0, 320:322], ones_sb[:, :32], fcb_sb[:], start=True, stop=True
            )
            nc.tensor.matmul(cr_ps[:, 1, :BL], rep_ap, eye_sb[:], start=True, stop=True)
            b0r = cpool.tile([128, H1], F32)
            nc.scalar.copy(b0r[:], cr_ps[:, 0, 0:H1])
            b1r = cpool.tile([128, H2], F32)
            nc.scalar.copy(b1r[:], cr_ps[:, 0, H1 : H1 + H2])
            fcwr = cpool.tile([BL, 2, OUTW], F32)
            nc.scalar.copy(fcwr[:], cr_ps[:32, 0, 128 : 128 + 2 * OUTW].rearrange("p (c k) -> p c k", c=2))
            fcbr = cpool.tile([BL, 2], F32)
            nc.scalar.copy(fcbr[:], cr_ps[:32, 0, 320:322])
            s4 = cpool.tile([128, BL], F32)
            nc.scalar.copy(s4[:], cr_ps[:, 1, :BL])

            # accumulation targets: slot t = bd-tile t
            xpc = cpool.tile([128, 4, HN], F32)
            ctc = cpool.tile([128, 4, OUTW], F32)


            def lhs(t, ko):
                # stationary operand [m-chunk 128, 128 bd cols of tile t]
                return v[:, ko, 4 * t : 4 * (t + 1), :]

            # ---- layer 1 + interleaved layer 2 ----
            def l1_step(j):
                a_j = apool.tile([128, 2, 512], F32R, tag="a", name="a_j")
                nc.sync.dma_start(
                    a_j[:, 0, 0:384], at3[:, 0, 512 * j : 512 * j + 384]
                )
                nc.sync.dma_start(a_j[:, 1, :], at3[:, 1, 512 * j : 512 * (j + 1)])
                for tp in range(2):  # tile pair (2*tp, 2*tp+1)
                    ps = ppool.tile([128, 2, 512], F32, tag="u", name="ps")
                    for b_ in range(2):
                        t = 2 * tp + b_
                        # triangular-packed weights: ko=0 rows only touch
                        # h<192... columns [0:384] (rest are zeros). Full-width
                        # ko=1 matmul first so every element gets a start=True
                        # write, then the narrower ko=0 accumulate.
                        nc.tensor.matmul(
                            ps[:, b_, :], lhs(t, 1), a_j[:, 1, :],
                            start=True, stop=False,
                        )
                        nc.tensor.matmul(
                            ps[:, b_, 0:384], lhs(t, 0), a_j[:, 0, 0:384],
                            start=False, stop=True, skip_group_check=True,
                        )
                    tmp = spool.tile([128, 2, 2, 256], F32, tag="tmp", name="tmp")
                    nc.vector.tensor_tensor(
                        out=tmp[:],
                        in0=ps.rearrange("p b (oh m) -> p b oh m", m=256),
                        in1=vT[:, 2 * tp : 2 * tp + 2, None, :].to_broadcast(
                            [128, 2, 2, 256]
                        ),
                        op=mybir.AluOpType.mult,
                    )
                    if (j % 3) == 2 or j in (30, 31):
                        # DVE grouped reduce: out[b_, oh] -> (t=2tp+b_, o=2j+oh)
                        tgt4 = (
                            xpc[:, 2 * tp : 2 * tp + 2, 2 * j : 2 * j + 2]
                            if 2 * j < HN
                            else ctc[:, 2 * tp : 2 * tp + 2, 2 * j - HN : 2 * j - HN + 2]
                        )
                        nc.vector.tensor_reduce(
                            out=tgt4,
                            in_=tmp[:],
                            axis=mybir.AxisListType.X,
                            op=mybir.AluOpType.add,
                        )
                    else:
                        for b_ in range(2):
                            t = 2 * tp + b_
                            for oh in range(2):
                                o = 2 * j + oh
                                tgt = (
                                    xpc[:, t, o : o + 1]
                                    if o < HN
                                    else ctc[:, t, o - HN : o - HN + 1]
                                )
                                dump = dpool.tile([128, 256], F32, tag="dump", name="dump")
                                nc.scalar.activation(
                                    dump[:],
                                    tmp[:, b_, oh, :],
                                    mybir.ActivationFunctionType.Copy,
                                    bias=0.0,
                                    scale=1.0,
                                    accum_out=tgt,
                                )

            def l2_step(jj):
                w_jj = apool.tile([128, 2, 512], F32R, tag="a", name="w_jj")
                nc.sync.dma_start(w_jj[:], w1t3[:, :, 512 * jj : 512 * (jj + 1)])
                for tp in range(2):
                    ps2 = ppool.tile([128, 2, 512], F32, tag="u", name="ps2")
                    for b_ in range(2):
                        t = 2 * tp + b_
                        for ko in range(2):
                            nc.tensor.matmul(
                                ps2[:, b_, :],
                                lhs(t, ko),
                                w_jj[:, ko, :],
                                start=(ko == 0),
                                stop=(ko == 1),
                            )
                    tmp2 = spool.tile([128, 2, 16, HN], F32, tag="tmp2", name="tmp2")
                    nc.vector.tensor_tensor(
                        out=tmp2[:],
                        in0=ps2.rearrange("p b (g h) -> p b g h", h=HN),
                        in1=xpc[:, 2 * tp : 2 * tp + 2, None, :].to_broadcast(
                            [128, 2, 16, HN]
                        ),
                        op=mybir.AluOpType.mult,
                    )
                    for b_ in range(2):
                        t = 2 * tp + b_
                        nc.vector.tensor_reduce(
                            out=ctc[:, t, HN + 16 * jj : HN + 16 * (jj + 1)],
                            in_=tmp2[:, b_],
                            axis=mybir.AxisListType.X,
                            op=mybir.AluOpType.add,
                        )

            for j in range(NJ // 2):
                l1_step(j)
            # hidden half accumulated -> bias + relu Xp for layer 2
            nc.vector.tensor_tensor(
                out=xpc[:],
                in0=xpc[:],
                in1=b0r[:, None, :HN].to_broadcast([128, 4, HN]),
                op=mybir.AluOpType.add,
            )
            nc.vector.tensor_scalar_max(xpc[:], xpc[:], 0.0)
            for j in range(NJ // 2, NJ):
                l1_step(j)
                if j % 4 == 1:
                    l2_step((j - NJ // 2) // 4)

            # bias + relu: ctc cols 0:32 are X1[32:64] (need b0[32:]), cols
            # 32:96 are X2 (need b1)
            nc.vector.tensor_tensor(
                out=ctc[:, :, :HN],
                in0=ctc[:, :, :HN],
                in1=b0r[:, None, HN:].to_broadcast([128, 4, HN]),
                op=mybir.AluOpType.add,
            )
            nc.vector.tensor_tensor(
                out=ctc[:, :, HN:],
                in0=ctc[:, :, HN:],
                in1=b1r[:, None, :].to_broadcast([128, 4, H2]),
                op=mybir.AluOpType.add,
            )
            nc.vector.tensor_scalar_max(ctc[:], ctc[:], 0.0)

            # ---- d-sum + fc ----
            psf = ppool.tile([BL, OUTW], F32, tag="u", name="psf")
            for t in range(4):
                nc.tensor.matmul(psf[:], s4[:], ctc[:, t, :], start=(t == 0), stop=(t == 3))
            cin = spool.tile([BL, OUTW], F32, tag="cin")
            nc.scalar.copy(cin[:], psf[:])
            y_sb = spool.tile([BL, 2], F32, tag="ysb")
            prod = spool.tile([BL, 2, OUTW], F32, tag="prod")
            nc.vector.tensor_tensor(
                out=prod[:],
                in0=cin[:, None, :].to_broadcast([BL, 2, OUTW]),
                in1=fcwr[:],
                op=mybir.AluOpType.mult,
            )
            nc.vector.tensor_reduce(
                out=y_sb[:],
                in_=prod[:],
                axis=mybir.AxisListType.X,
                op=mybir.AluOpType.add,
            )
            nc.vector.tensor_tensor(
                out=y_sb[:], in0=y_sb[:], in1=fcbr[:], op=mybir.AluOpType.add
            )
            nc.sync.dma_start(y_d[:], y_sb[:])

    nc.finalize()
    return nc


def kernel(x, emb, W0, b0, W1, b1, fcW, fcb):
    x = np.ascontiguousarray(x, dtype=np.float32)
    emb = np.ascontiguousarray(emb, dtype=np.float32)

    # host-side: symmetrize the quadratic form and pack upper-triangular
    # (zero for m < h, doubled off-diagonal), then permute [o,h,m] -> [m,(o,h)]
    W0r_ = W0.reshape(H1, M, M).astype(np.float64)
    S = 0.5 * (W0r_ + W0r_.transpose(0, 2, 1))
    iu = np.triu_indices(M, 1)
    Tri = np.zeros_like(S)
    Tri[:, np.arange(M), np.arange(M)] = S[:, np.arange(M), np.arange(M)]
    Tri[:, iu[0], iu[1]] = 2.0 * S[:, iu[0], iu[1]]
    at = np.ascontiguousarray(
        Tri.transpose(2, 0, 1).reshape(M, H1 * M).astype(np.float32)
    )
    w1t = np.ascontiguousarray(
        W1.reshape(H2, HN, M).transpose(2, 0, 1).reshape(M, H2 * HN).astype(np.float32)
    )
    embt = np.ascontiguousarray(emb.T)
    eye32 = np.eye(BL, dtype=np.float32)
    emb_arr = np.ascontiguousarray(
        emb.reshape(2, 128, D).transpose(1, 0, 2).reshape(128, 2 * D)
    )
    sel = np.zeros((D, 4, 128), dtype=np.float32)
    for t in range(4):
        for p in range(128):
            sel[4 * t + p // 32, t, p] = 1.0
    sel = sel.reshape(D, 4 * 128)
    ones = np.ones((1, 128), dtype=np.float32)

    shared = {
        "at": at,
        "w1t": w1t,
        "emb": emb_arr,
        "embt": embt,
        "sel": sel,
        "ones": ones,
        "b0": np.ascontiguousarray(b0.reshape(1, H1).astype(np.float32)),
        "b1": np.ascontiguousarray(b1.reshape(1, H2).astype(np.float32)),
        "fcw": np.ascontiguousarray(fcW.astype(np.float32)),
        "fcb": np.ascontiguousarray(fcb.reshape(1, 2).astype(np.float32)),
        "eye32": eye32,
    }
    in_maps = []
    for c in range(N_CORES):
        xs = np.ascontiguousarray(x[BL * c : BL * (c + 1)])
        m = dict(shared)
        m["xs"] = xs
        m["xt"] = np.ascontiguousarray(
            xs.T.reshape(2, 128, BL).transpose(1, 0, 2).reshape(128, 2 * BL)
        )
        in_maps.append(m)

    if "nc" not in _CACHE:
        _CACHE["nc"] = _build_nc()
    global _last_in_maps
    _last_in_maps = in_maps
    res = run_bass_kernel_spmd(_CACHE["nc"], in_maps, core_ids=list(range(N_CORES)))
    return np.concatenate([r["y"] for r in res.results], axis=0)
